# revision 1
# baseline (speedup 1.0000x reference)
import sys

sys.path.insert(0, "/opt/trn_rl_repo")

from contextlib import ExitStack

import ml_dtypes
import numpy as np

import concourse.bass as bass
import concourse.mybir as mybir
import concourse.tile as tile
from concourse import bacc, bass_utils

N, OBS, ENC, ACT, K = 16384, 512, 512, 64, 8
ALPHA = 1.0
NCORES = 8
R = N // NCORES  # rows per core
P = 128
NT = R // P  # n-tiles per core
NB = R // 512  # 512-wide n blocks
NH = ENC // P
NO = OBS // P
F32 = mybir.dt.float32
AX = mybir.AluOpType


def build_nc(mm_dtype=F32):
    # Bacc (not bass.Bass): its finalize() runs move_matmul_waits_to_ldweights
    # + generate_event_semaphores, required by TRN2's 1-wait-per-inst limit.
    nc = bacc.Bacc("TRN2", target_bir_lowering=False)
    x0t = nc.declare_dram_parameter("x0t", [OBS, R], mm_dtype, isOutput=False)
    x1t = nc.declare_dram_parameter("x1t", [OBS, R], mm_dtype, isOutput=False)
    ut = nc.declare_dram_parameter("ut", [ACT, R], mm_dtype, isOutput=False)
    wet = nc.declare_dram_parameter("wet", [OBS, ENC], mm_dtype, isOutput=False)
    at = nc.declare_dram_parameter("at", [K, ENC, ENC], mm_dtype, isOutput=False)
    ball = nc.declare_dram_parameter("ball", [K, ACT, ENC], mm_dtype, isOutput=False)
    cwt = nc.declare_dram_parameter("cwt", [ENC, K], mm_dtype, isOutput=False)
    cb = nc.declare_dram_parameter("cb", [1, K], F32, isOutput=False)
    loss = nc.declare_dram_parameter("loss_out", [1, 1], F32, isOutput=True)

    with tile.TileContext(nc) as tc, ExitStack() as ctx:
        const = ctx.enter_context(tc.tile_pool(name="const", bufs=1))
        stream = ctx.enter_context(tc.tile_pool(name="stream", bufs=2))
        dwork = ctx.enter_context(tc.tile_pool(name="dwork", bufs=3))
        psumA = ctx.enter_context(tc.tile_pool(name="psumA", bufs=4, space="PSUM"))
        psumS = ctx.enter_context(tc.tile_pool(name="psumS", bufs=2, space="PSUM"))

        # --- resident weights/activations ---
        wet_sb = const.tile([P, NO, ENC], mm_dtype)  # [o%128, o//128, h]
        nc.sync.dma_start(wet_sb[:], wet.rearrange("(c p) h -> p c h", p=P))
        ball_sb = const.tile([ACT, K, ENC], mm_dtype)  # [a, k, e]
        nc.sync.dma_start(ball_sb[:], ball.rearrange("k a e -> a k e"))
        cwt_sb = const.tile([P, NH, K], mm_dtype)  # [h%128, h//128, k]
        nc.sync.dma_start(cwt_sb[:], cwt.rearrange("(c p) k -> p c k", p=P))
        cb128 = const.tile([P, K], F32)
        nc.sync.dma_start(cb128[:], bass.AP(tensor=cb, offset=0, ap=[[0, P], [1, K]]))
        ut_sb = const.tile([ACT, R], mm_dtype)  # [a, n]
        nc.sync.dma_start(ut_sb[:], ut[:])

        x0et = const.tile([P, NH, R], mm_dtype)  # [h%128, h//128, n]
        x1e = const.tile([P, NT, ENC], F32)  # [n%128, n//128, e]

        iota_i = const.tile([P, K], mybir.dt.int32)
        nc.gpsimd.iota(iota_i[:], pattern=[[1, K]], base=0, channel_multiplier=0)
        iota_f = const.tile([P, K], F32)
        nc.scalar.copy(iota_f[:], iota_i[:])
        oh_all = const.tile([P, NT * K], F32)
        sq_all = const.tile([P, NT * K], F32)
        acc = const.tile([P, NT], F32)

        x0t_r = x0t.rearrange("(c p) n -> p c n", p=P)
        x1t_r = x1t.rearrange("(c p) n -> p c n", p=P)
        at_r = at.rearrange("k (c p) e -> p k c e", p=P)

        # --- phase A: encode (X0e^T and X1e) ---
        for nb in range(NB):
            ns = slice(nb * 512, (nb + 1) * 512)
            x0c = stream.tile([P, NO, 512], mm_dtype, name="x0c")
            nc.sync.dma_start(x0c[:], x0t_r[:, :, ns])
            x1c = stream.tile([P, NO, 512], mm_dtype, name="x1c")
            nc.sync.dma_start(x1c[:], x1t_r[:, :, ns])
            for hc in range(NH):
                pt = psumA.tile([P, 512], F32, name="pA")
                for oc in range(NO):
                    nc.tensor.matmul(
                        pt[:],
                        wet_sb[:, oc, hc * P : (hc + 1) * P],
                        x0c[:, oc, :],
                        start=(oc == 0),
                        stop=(oc == NO - 1),
                    )
                nc.scalar.copy(x0et[:, hc, ns], pt[:])
            for j in range(4):
                nt = nb * 4 + j
                pt = psumA.tile([P, 512], F32, name="pA")
                for oc in range(NO):
                    nc.tensor.matmul(
                        pt[:],
                        x1c[:, oc, j * P : (j + 1) * P],
                        wet_sb[:, oc, :],
                        start=(oc == 0),
                        stop=(oc == NO - 1),
                    )
                nc.scalar.copy(x1e[:, nt, :], pt[:])

        # --- phase B: router logits, argmax, one-hot ---
        for nt in range(NT):
            nts = slice(nt * P, (nt + 1) * P)
            pl = psumS.tile([P, K], F32, name="pl")
            for hc in range(NH):
                nc.tensor.matmul(
                    pl[:],
                    x0et[:, hc, nts],
                    cwt_sb[:, hc, :],
                    start=(hc == 0),
                    stop=(hc == NH - 1),
                )
            lg = dwork.tile([P, K], F32, name="lg")
            nc.vector.tensor_tensor(lg[:], pl[:], cb128[:], AX.add)
            mx = dwork.tile([P, K], F32, name="mx")
            ix = dwork.tile([P, K], mybir.dt.uint32, name="ix")
            nc.vector.max_with_indices(mx[:], ix[:], lg[:])
            ixf = dwork.tile([P, 1], F32, name="ixf")
            nc.scalar.copy(ixf[:], ix[:, 0:1])
            nc.vector.tensor_scalar(
                oh_all[:, nt * K : (nt + 1) * K],
                iota_f[:],
                ixf[:],
                None,
                op0=AX.is_equal,
            )

        # --- phase C: per-expert preds, squared error ---
        for k in range(K):
            atk = stream.tile([P, NH, ENC], mm_dtype, name="atk")
            nc.sync.dma_start(atk[:], at_r[:, k, :, :])
            for nt in range(NT):
                nts = slice(nt * P, (nt + 1) * P)
                pd = psumA.tile([P, 512], F32, name="pA")
                for hc in range(NH):
                    nc.tensor.matmul(
                        pd[:],
                        x0et[:, hc, nts],
                        atk[:, hc, :],
                        start=(hc == 0),
                        stop=False,
                    )
                nc.tensor.matmul(
                    pd[:], ut_sb[:, nts], ball_sb[:, k, :], start=False, stop=True
                )
                # GPSIMD cannot read PSUM and TensorScalarPtr is illegal on Pool:
                # vector does the subtract (PSUM->SBUF), ACT does square+accum.
                df = dwork.tile([P, ENC], F32, name="df")
                nc.vector.tensor_tensor(df[:], x1e[:, nt, :], pd[:], AX.subtract)
                sj = dwork.tile([P, ENC], F32, name="sj")
                nc.scalar.activation(
                    sj[:],
                    df[:],
                    mybir.ActivationFunctionType.Square,
                    accum_out=sq_all[:, nt * K + k : nt * K + k + 1],
                )

        # --- phase D: select routed expert's sq, accumulate ---
        for nt in range(NT):
            ks = slice(nt * K, (nt + 1) * K)
            sel = dwork.tile([P, K], F32, name="sel")
            nc.vector.scalar_tensor_tensor(
                sel[:],
                sq_all[:, ks],
                1.0,
                oh_all[:, ks],
                op0=AX.mult,
                op1=AX.mult,
                accum_out=acc[:, nt : nt + 1],
            )

        out_sb = const.tile([1, 1], F32)
        nc.gpsimd.tensor_reduce(
            out_sb[:], acc[:], axis=mybir.AxisListType.XYZWC, op=AX.add
        )
        nc.sync.dma_start(loss[:], out_sb[:])

    nc.finalize()
    return nc


T_TILES = 136  # 17408 padded slots: 16384 rows + <=127 pad/expert + global pad
NT2 = T_TILES // NCORES  # 17 tiles per core
R2 = NT2 * P  # 2176 rows per core
CHUNKS = [(0, 512), (512, 512), (1024, 512), (1536, 512), (2048, 128)]


def build_nc_routed(mm_dtype):
    # Encoder folded into weights on host: pred - x1e =
    #   x0 @ (W^T A_k^T) + u @ B_k - x1 @ W^T  -> 9 matmuls into one PSUM bank,
    # ACT squares straight from PSUM. Square kills the sign, so wetn = -W^T.
    nc = bacc.Bacc("TRN2", target_bir_lowering=False)
    x0t = nc.declare_dram_parameter("x0t", [OBS, R2], mm_dtype, isOutput=False)
    x1t = nc.declare_dram_parameter("x1t", [OBS, R2], mm_dtype, isOutput=False)
    ut = nc.declare_dram_parameter("ut", [ACT, R2], mm_dtype, isOutput=False)
    wetn = nc.declare_dram_parameter("wetn", [OBS, ENC], mm_dtype, isOutput=False)
    atb = nc.declare_dram_parameter("atb", [P, NT2 * NO, ENC], mm_dtype, isOutput=False)
    ballb = nc.declare_dram_parameter("ballb", [ACT, NT2, ENC], mm_dtype, isOutput=False)
    loss = nc.declare_dram_parameter("loss_out", [1, 1], F32, isOutput=True)

    with tile.TileContext(nc) as tc, ExitStack() as ctx:
        const = ctx.enter_context(tc.tile_pool(name="const", bufs=1))
        stream = ctx.enter_context(tc.tile_pool(name="stream", bufs=NT2))
        dwork = ctx.enter_context(tc.tile_pool(name="dwork", bufs=3))
        psumA = ctx.enter_context(tc.tile_pool(name="psumA", bufs=8, space="PSUM"))

        wetn_sb = const.tile([P, NO, ENC], mm_dtype)
        nc.sync.dma_start(wetn_sb[:], wetn.rearrange("(c p) h -> p c h", p=P))
        ut_sb = const.tile([ACT, R2], mm_dtype)
        nc.sync.dma_start(ut_sb[:], ut[:])
        ballb_sb = const.tile([ACT, NT2, ENC], mm_dtype)
        nc.sync.dma_start(ballb_sb[:], ballb[:])

        x0t_r = x0t.rearrange("(c p) n -> p c n", p=P)
        x1t_r = x1t.rearrange("(c p) n -> p c n", p=P)
        x0t_sb = const.tile([P, NO, R2], mm_dtype)
        x1t_sb = const.tile([P, NO, R2], mm_dtype)
        H = R2 // 2
        for h in range(2):
            hs = slice(h * H, (h + 1) * H)
            nc.sync.dma_start(x0t_sb[:, :, hs], x0t_r[:, :, hs])
            nc.sync.dma_start(x1t_sb[:, :, hs], x1t_r[:, :, hs])

        acc = const.tile([P, NT2], F32)

        for nt in range(NT2):
            nts = slice(nt * P, (nt + 1) * P)
            atk = stream.tile([P, NO, ENC], mm_dtype, name="atk")
            nc.sync.dma_start(atk[:], atb[:, nt * NO : (nt + 1) * NO, :])
            pd = psumA.tile([P, ENC], F32, name="pA")
            for oc in range(NO):
                nc.tensor.matmul(
                    pd[:],
                    x0t_sb[:, oc, nts],
                    atk[:, oc, :],
                    start=(oc == 0),
                    stop=False,
                )
            nc.tensor.matmul(
                pd[:],
                ut_sb[:, nts],
                ballb_sb[:, nt, :],
                start=False,
                stop=False,
            )
            for oc in range(NO):
                nc.tensor.matmul(
                    pd[:],
                    x1t_sb[:, oc, nts],
                    wetn_sb[:, oc, :],
                    start=False,
                    stop=(oc == NO - 1),
                )
            sj = dwork.tile([P, ENC], F32, name="sj")
            nc.scalar.activation(
                sj[:],
                pd[:],
                mybir.ActivationFunctionType.Square,
                accum_out=acc[:, nt : nt + 1],
            )

        out_sb = const.tile([1, 1], F32)
        nc.gpsimd.tensor_reduce(
            out_sb[:], acc[:], axis=mybir.AxisListType.XYZWC, op=AX.add
        )
        nc.sync.dma_start(loss[:], out_sb[:])

    nc.finalize()
    return nc


_NC_CACHE = {}
MM_BF16 = True
ROUTED = True


def _get_nc():
    key = ("routed" if ROUTED else "dense", MM_BF16)
    if key not in _NC_CACHE:
        # bf16: 1 cyc/row on PE (f32r measured ~2 due to 4B SBUF moving-read cap)
        dt = mybir.dt.bfloat16 if MM_BF16 else mybir.dt.float32r
        _NC_CACHE[key] = build_nc_routed(dt) if ROUTED else build_nc(dt)
    return _NC_CACHE[key]


def _route_slots(X0, W_enc, C_w, C_b):
    # f64 router on host: argmax(X0 @ W_enc.T @ C_w.T + C_b) per row
    m = (C_w.astype(np.float64) @ W_enc.astype(np.float64)).T  # [OBS, K]
    logits = X0.astype(np.float64) @ m + C_b.astype(np.float64)
    inds = np.argmax(logits, axis=1)
    rows_l, eids = [], []
    for k in range(K):
        rk = np.nonzero(inds == k)[0]
        pad = (-len(rk)) % P
        rows_l.append(rk)
        rows_l.append(np.full(pad, -1, np.int64))
        eids += [k] * ((len(rk) + pad) // P)
    rows = np.concatenate(rows_l)
    rows = np.concatenate([rows, np.full(T_TILES * P - len(rows), -1, np.int64)])
    eids += [0] * (T_TILES - len(eids))
    return rows, np.asarray(eids)


def make_in_maps(X1, X0, U, W_enc, A_all, B_rest, C_w, C_b):
    mm_np = ml_dtypes.bfloat16 if MM_BF16 else np.float32
    wet = np.ascontiguousarray(W_enc.T).astype(mm_np)  # [OBS, ENC]
    at = A_all.transpose(0, 2, 1).astype(mm_np)  # [K, h, e]
    b0 = np.eye(ENC, dtype=np.float32)[:ACT]
    ball = np.concatenate([b0[None], B_rest], axis=0).astype(mm_np)  # [K, a, e]

    if not ROUTED:
        cwt = np.ascontiguousarray(C_w.T).astype(mm_np)
        cb = np.ascontiguousarray(C_b.reshape(1, K))
        in_maps = []
        for i in range(NCORES):
            rs = slice(i * R, (i + 1) * R)
            in_maps.append(
                {
                    "x0t": np.ascontiguousarray(X0[rs].T).astype(mm_np),
                    "x1t": np.ascontiguousarray(X1[rs].T).astype(mm_np),
                    "ut": np.ascontiguousarray(U[rs].T).astype(mm_np),
                    "wet": wet,
                    "at": at,
                    "ball": ball,
                    "cwt": cwt,
                    "cb": cb,
                }
            )
        return in_maps

    rows, eids = _route_slots(X0, W_enc, C_w, C_b)
    safe = np.clip(rows, 0, None)
    zero = (rows < 0)[:, None]

    def take0(M):
        out = M[safe].astype(mm_np)
        out[np.broadcast_to(zero, out.shape)] = 0
        return out

    X0s, X1s, Us = take0(X0), take0(X1), take0(U)
    wT = W_enc.T.astype(np.float32)  # [OBS, ENC]
    ae = (wT[None] @ A_all.transpose(0, 2, 1).astype(np.float32)).astype(mm_np)
    wetn = np.ascontiguousarray(-wT).astype(mm_np)
    in_maps = []
    for i in range(NCORES):
        sl = slice(i * R2, (i + 1) * R2)
        te = eids[i * NT2 : (i + 1) * NT2]
        atb = ae[te].reshape(NT2, NO, P, ENC).transpose(2, 0, 1, 3)
        in_maps.append(
            {
                "x0t": np.ascontiguousarray(X0s[sl].T),
                "x1t": np.ascontiguousarray(X1s[sl].T),
                "ut": np.ascontiguousarray(Us[sl].T),
                "wetn": wetn,
                "atb": np.ascontiguousarray(atb).reshape(P, NT2 * NO, ENC),
                "ballb": np.ascontiguousarray(ball[te].transpose(1, 0, 2)),
            }
        )
    return in_maps


def kernel(X1, X0, U, W_enc, A_all, B_rest, C_w, C_b):
    nc = _get_nc()
    in_maps = make_in_maps(X1, X0, U, W_enc, A_all, B_rest, C_w, C_b)
    res = bass_utils.run_bass_kernel_spmd(nc, in_maps, list(range(NCORES)))
    total = sum(float(r["loss_out"][0, 0]) for r in res.results)
    return np.float32(ALPHA * total / (ENC * N))



# revision 6
# speedup vs baseline: 1.6798x; 1.6798x over previous
import sys

sys.path.insert(0, "/opt/trn_rl_repo")

from contextlib import ExitStack

import ml_dtypes
import numpy as np

import concourse.bass as bass
import concourse.mybir as mybir
import concourse.tile as tile
from concourse import bacc, bass_utils

N, OBS, ENC, ACT, K = 16384, 512, 512, 64, 8
ALPHA = 1.0
NCORES = 8
P = 128
F32 = mybir.dt.float32
FP8 = mybir.dt.float8e4
NP_FP8 = ml_dtypes.float8_e4m3
AX = mybir.AluOpType
DR = mybir.MatmulPerfMode.DoubleRow

# One expert per core. Per routed row n with expert k:
#   diff = x0 @ (W^T A_k^T) + u @ B_k - x1 @ W^T ; loss += ||diff||^2
# Contraction z = [x0(512); u(64)+pad(64); x1(512); zeros(128)] = 10 chunks of
# 128, processed as 5 fp8 DoubleRow matmuls (2 chunks each) into one PSUM
# bank per 128-row tile. Weights (chunk-matched, x16 scaled for fp8 range)
# are SBUF-resident; only this core's expert is ever loaded.
NCH = 10  # contraction chunks incl. u-pad and trailing zero pad
ZCH = 9  # chunks with host-provided data (zero pad chunk is memset)
WSCALE = 16.0


def build_nc(NT3, NBLK):
    R3 = NBLK * 512
    nc = bacc.Bacc("TRN2", target_bir_lowering=False)
    zb = nc.declare_dram_parameter("zb", [NBLK, ZCH * P, 512], FP8, isOutput=False)
    wtb = nc.declare_dram_parameter("wtb", [NCH * P, ENC], FP8, isOutput=False)
    loss = nc.declare_dram_parameter("loss_out", [P, (NT3 + 1) // 2], F32, isOutput=True)

    with tile.TileContext(nc) as tc, ExitStack() as ctx:
        const = ctx.enter_context(tc.tile_pool(name="const", bufs=1))
        dwork = ctx.enter_context(tc.tile_pool(name="dwork", bufs=4))
        psumA = ctx.enter_context(tc.tile_pool(name="psumA", bufs=4, space="PSUM"))

        w_sb = const.tile([P, NCH, ENC], FP8)
        z_sb = const.tile([P, NCH, R3], FP8)
        acc = const.tile([P, (NT3 + 1) // 2], F32)

        wtb_r = wtb.rearrange("(c p) e -> p c e", p=P)
        zb_r = zb.rearrange("b (c p) n -> p b c n", p=P)

        # zero pad chunk (data half of the 5th DoubleRow pair)
        nc.gpsimd.memset(z_sb[:, ZCH : ZCH + 1, :], 0.0)
        # weights first, in consumption-order pair pieces so tile 0 can start
        # after only pair 0 has landed
        for j in range(NCH // 2):
            nc.sync.dma_start(w_sb[:, 2 * j : 2 * j + 2, :], wtb_r[:, 2 * j : 2 * j + 2, :])
        for b in range(NBLK):
            bs = slice(b * 512, (b + 1) * 512)
            for j in range(NCH // 2):
                lo, hi = 2 * j, min(2 * j + 2, ZCH)
                nc.sync.dma_start(z_sb[:, lo:hi, bs], zb_r[:, b, lo:hi, :])

        # Row tiles are processed in pairs sharing a 2-bank PSUM tile. The
        # square+accumulate alternates between ACT (one 1024-wide op per
        # pair, amortizing its fixed overheads) and DVE (PSUM->SBUF copy
        # then square from SBUF: DVE cannot dual-read PSUM) so neither
        # engine becomes the critical path next to the PE.
        npairs = (NT3 + 1) // 2
        for pi in range(npairs):
            tiles = [t for t in (2 * pi, 2 * pi + 1) if t < NT3]
            pd = psumA.tile([P, 2, ENC], F32, name="pA")
            for i, nt in enumerate(tiles):
                nts = slice(nt * P, (nt + 1) * P)
                for j in range(NCH // 2):
                    nc.tensor.matmul(
                        pd[:, i, :],
                        z_sb[:, 2 * j : 2 * j + 2, nts],
                        w_sb[:, 2 * j : 2 * j + 2, :],
                        start=(j == 0),
                        stop=(j == NCH // 2 - 1),
                        perf_mode=DR,
                    )
            if pi % 3 != 2 or len(tiles) == 1:
                sj = dwork.tile([P, len(tiles), ENC], F32, name="sj")
                nc.scalar.activation(
                    sj[:],
                    pd[:, 0 : len(tiles), :],
                    mybir.ActivationFunctionType.Square,
                    accum_out=acc[:, pi : pi + 1],
                )
            else:
                sv = dwork.tile([P, 2, ENC], F32, name="sv")
                nc.vector.tensor_scalar(sv[:], pd[:], 1.0, None, op0=AX.mult)
                sj = dwork.tile([P, 2, ENC], F32, name="sj")
                nc.vector.scalar_tensor_tensor(
                    sj[:],
                    sv[:],
                    1.0,
                    sv[:],
                    op0=AX.mult,
                    op1=AX.mult,
                    accum_out=acc[:, pi : pi + 1],
                )
        nc.sync.dma_start(loss[:], acc[:, 0:npairs])

    nc.finalize()
    return nc


_NC_CACHE = {}


def _get_nc(NT3=None, NBLK=None):
    if NT3 is None:
        key = next(reversed(_NC_CACHE))
    else:
        key = (NT3, NBLK)
        if key not in _NC_CACHE:
            _NC_CACHE[key] = build_nc(*key)
    return _NC_CACHE[key]


def _route(X0, W_enc, C_w, C_b):
    # f64 router on host: argmax(X0 @ W_enc.T @ C_w.T + C_b) per row
    m = (C_w.astype(np.float64) @ W_enc.astype(np.float64)).T  # [OBS, K]
    logits = X0.astype(np.float64) @ m + C_b.astype(np.float64)
    return np.argmax(logits, axis=1)


def _shapes_for(counts):
    NT3 = max(1, -(-int(counts.max()) // P))
    NBLK = -(-NT3 * P // 512)
    return NT3, NBLK


def make_in_maps(X1, X0, U, W_enc, A_all, B_rest, C_w, C_b):
    inds = _route(X0, W_enc, C_w, C_b)
    counts = np.bincount(inds, minlength=K)
    NT3, NBLK = _shapes_for(counts)
    R3 = NBLK * 512

    wT = W_enc.T.astype(np.float32)  # [OBS, ENC]
    ae = wT[None] @ A_all.transpose(0, 2, 1).astype(np.float32)  # [K, OBS, ENC]
    b0 = np.eye(ENC, dtype=np.float32)[:ACT]
    ball = np.concatenate([b0[None], B_rest.astype(np.float32)], axis=0)

    in_maps = []
    for k in range(K):
        rk = np.nonzero(inds == k)[0]
        c = len(rk)
        Z = np.zeros((ZCH * P, R3), dtype=np.float32)
        Z[0:OBS, :c] = X0[rk].T
        Z[OBS : OBS + ACT, :c] = U[rk].T
        Z[OBS + P : OBS + P + OBS, :c] = X1[rk].T
        zq = Z.astype(NP_FP8)
        zblk = np.ascontiguousarray(
            zq.reshape(ZCH, P, NBLK, 512).transpose(2, 0, 1, 3)
        ).reshape(NBLK, ZCH * P, 512)

        Wt = np.zeros((NCH * P, ENC), dtype=np.float32)
        Wt[0:OBS] = ae[k] * WSCALE
        Wt[OBS : OBS + ACT] = ball[k] * WSCALE
        Wt[OBS + P : OBS + P + OBS] = -wT * WSCALE
        in_maps.append({"zb": zblk, "wtb": Wt.astype(NP_FP8)})
    return in_maps, NT3, NBLK


def kernel(X1, X0, U, W_enc, A_all, B_rest, C_w, C_b):
    in_maps, NT3, NBLK = make_in_maps(X1, X0, U, W_enc, A_all, B_rest, C_w, C_b)
    nc = _get_nc(NT3, NBLK)
    res = bass_utils.run_bass_kernel_spmd(nc, in_maps, list(range(NCORES)))
    total = sum(float(r["loss_out"].sum()) for r in res.results)
    return np.float32(ALPHA * total / (WSCALE * WSCALE * ENC * N))


# revision 9
# speedup vs baseline: 1.8344x; 1.0921x over previous
import sys

sys.path.insert(0, "/opt/trn_rl_repo")

from contextlib import ExitStack

import ml_dtypes
import numpy as np

import concourse.bass as bass
import concourse.mybir as mybir
import concourse.tile as tile
from concourse import bacc, bass_utils

N, OBS, ENC, ACT, K = 16384, 512, 512, 64, 8
ALPHA = 1.0
NCORES = 8
P = 128
F32 = mybir.dt.float32
FP8 = mybir.dt.float8e4
NP_FP8 = ml_dtypes.float8_e4m3
AX = mybir.AluOpType
DR = mybir.MatmulPerfMode.DoubleRow

# One expert per core. Per routed row n with expert k:
#   diff = x0 @ (W^T A_k^T) + u @ B_k - x1 @ W^T ; loss += ||diff||^2
# Contraction z = [x0(512); u(64)+pad(64); x1(512); zeros(128)] = 10 chunks of
# 128, processed as 5 fp8 DoubleRow matmuls (2 chunks each) into one PSUM
# bank per 128-row tile. Weights (chunk-matched, x16 scaled for fp8 range)
# are SBUF-resident; only this core's expert is ever loaded.
#
# Layouts are per-partition contiguous on the host so each DMA moves 2-10KB
# packets per partition (the DMA engines are descriptor-rate limited at
# ~45ns/packet). Columns are split in two groups so compute can start after
# ~1/3 of the data has landed; within a supergroup of 8 row-tiles the PE
# sweeps chunk-pair-major so each sweep only depends on one z piece.
NCH = 10  # contraction chunks incl. u-pad and trailing zero pad
ZCH = 9  # chunks with host-provided data (zero pad chunk is memset)
WSCALE = 16.0
SG = 8  # row tiles per supergroup (8 PSUM banks)


def build_nc(NT3):
    tiles0 = min(NT3, SG)
    tiles1 = NT3 - tiles0
    G0, G1 = tiles0 * P, tiles1 * P
    nc = bacc.Bacc("TRN2", target_bir_lowering=False)
    z0 = nc.declare_dram_parameter("z0", [P, ZCH, G0], FP8, isOutput=False)
    z1 = (
        nc.declare_dram_parameter("z1", [P, ZCH, G1], FP8, isOutput=False)
        if tiles1
        else None
    )
    wtb = nc.declare_dram_parameter("wtb", [P, NCH, ENC], FP8, isOutput=False)
    npairs = (NT3 + 1) // 2
    loss = nc.declare_dram_parameter("loss_out", [P, npairs], F32, isOutput=True)

    with tile.TileContext(nc) as tc, ExitStack() as ctx:
        const = ctx.enter_context(tc.tile_pool(name="const", bufs=1))
        dwork = ctx.enter_context(tc.tile_pool(name="dwork", bufs=4))
        psumA = ctx.enter_context(tc.tile_pool(name="psumA", bufs=4, space="PSUM"))

        w_sb = const.tile([P, NCH, ENC], FP8, name="wsb")
        z_sb = [const.tile([P, NCH, G0], FP8, name="zsb0")]
        if tiles1:
            z_sb.append(const.tile([P, NCH, G1], FP8, name="zsb1"))
        acc = const.tile([P, npairs], F32, name="accsb")

        for zt in z_sb:
            nc.gpsimd.memset(zt[:, ZCH : ZCH + 1, :], 0.0)
        # issue order tracks first-supergroup consumption order
        nc.sync.dma_start(w_sb[:, 0:2, :], wtb[:, 0:2, :])
        nc.sync.dma_start(z_sb[0][:, 0:2, :], z0[:, 0:2, :])
        nc.sync.dma_start(w_sb[:, 2:NCH, :], wtb[:, 2:NCH, :])
        for lo, hi in ((2, 4), (4, 5), (5, 7), (7, 9)):
            nc.sync.dma_start(z_sb[0][:, lo:hi, :], z0[:, lo:hi, :])
        if tiles1:
            for lo, hi in ((0, 2), (2, 4), (4, 5), (5, 7), (7, 9)):
                nc.sync.dma_start(z_sb[1][:, lo:hi, :], z1[:, lo:hi, :])

        # supergroups of SG row tiles; within each, sweep chunk-pairs so the
        # j-th sweep only needs the j-th z/w pieces
        sgs = [list(range(s, min(s + SG, NT3))) for s in range(0, NT3, SG)]
        for sgi, tiles in enumerate(sgs):
            zt = z_sb[0] if sgi == 0 else z_sb[1]
            base = 0 if sgi == 0 else SG
            pds = [psumA.tile([P, 2, ENC], F32, name="pA") for _ in range((len(tiles) + 1) // 2)]
            for j in range(NCH // 2):
                for i, nt in enumerate(tiles):
                    nts = slice((nt - base) * P, (nt - base + 1) * P)
                    nc.tensor.matmul(
                        pds[i // 2][:, i % 2, :],
                        zt[:, 2 * j : 2 * j + 2, nts],
                        w_sb[:, 2 * j : 2 * j + 2, :],
                        start=(j == 0),
                        stop=(j == NCH // 2 - 1),
                        perf_mode=DR,
                    )
            for i, pd in enumerate(pds):
                pi = (tiles[0] + 2 * i) // 2
                width = min(2, len(tiles) - 2 * i)
                if i % 4 != 3 or width == 1:
                    sj = dwork.tile([P, width, ENC], F32, name="sj")
                    nc.scalar.activation(
                        sj[:],
                        pd[:, 0:width, :],
                        mybir.ActivationFunctionType.Square,
                        accum_out=acc[:, pi : pi + 1],
                    )
                else:
                    sv = dwork.tile([P, 2, ENC], F32, name="sv")
                    nc.vector.tensor_scalar(sv[:], pd[:], 1.0, None, op0=AX.mult)
                    sj = dwork.tile([P, 2, ENC], F32, name="sj")
                    nc.vector.scalar_tensor_tensor(
                        sj[:],
                        sv[:],
                        1.0,
                        sv[:],
                        op0=AX.mult,
                        op1=AX.mult,
                        accum_out=acc[:, pi : pi + 1],
                    )
        nc.sync.dma_start(loss[:], acc[:])

    nc.finalize()
    return nc


_NC_CACHE = {}


def _get_nc(NT3=None):
    if NT3 is None:
        key = next(reversed(_NC_CACHE))
    else:
        key = NT3
        if key not in _NC_CACHE:
            _NC_CACHE[key] = build_nc(key)
    return _NC_CACHE[key]


def _route(X0, W_enc, C_w, C_b):
    # f64 router on host: argmax(X0 @ W_enc.T @ C_w.T + C_b) per row
    m = (C_w.astype(np.float64) @ W_enc.astype(np.float64)).T  # [OBS, K]
    logits = X0.astype(np.float64) @ m + C_b.astype(np.float64)
    return np.argmax(logits, axis=1)


def make_in_maps(X1, X0, U, W_enc, A_all, B_rest, C_w, C_b):
    inds = _route(X0, W_enc, C_w, C_b)
    counts = np.bincount(inds, minlength=K)
    NT3 = max(2, -(-int(counts.max()) // P))
    R3 = NT3 * P
    tiles0 = min(NT3, SG)
    G0 = tiles0 * P

    wT = W_enc.T.astype(np.float32)  # [OBS, ENC]
    ae = wT[None] @ A_all.transpose(0, 2, 1).astype(np.float32)  # [K, OBS, ENC]
    b0 = np.eye(ENC, dtype=np.float32)[:ACT]
    ball = np.concatenate([b0[None], B_rest.astype(np.float32)], axis=0)

    in_maps = []
    for k in range(K):
        rk = np.nonzero(inds == k)[0]
        c = len(rk)
        Z = np.zeros((ZCH * P, R3), dtype=np.float32)
        Z[0:OBS, :c] = X0[rk].T
        Z[OBS : OBS + ACT, :c] = U[rk].T
        Z[OBS + P : OBS + P + OBS, :c] = X1[rk].T
        zq = np.asarray(Z, dtype=NP_FP8).reshape(ZCH, P, R3)
        im = {
            "z0": np.ascontiguousarray(zq[:, :, 0:G0].transpose(1, 0, 2)),
        }
        if NT3 > tiles0:
            im["z1"] = np.ascontiguousarray(zq[:, :, G0:R3].transpose(1, 0, 2))

        Wt = np.zeros((NCH * P, ENC), dtype=np.float32)
        Wt[0:OBS] = ae[k] * WSCALE
        Wt[OBS : OBS + ACT] = ball[k] * WSCALE
        Wt[OBS + P : OBS + P + OBS] = -wT * WSCALE
        im["wtb"] = np.ascontiguousarray(
            np.asarray(Wt, dtype=NP_FP8).reshape(NCH, P, ENC).transpose(1, 0, 2)
        )
        in_maps.append(im)
    return in_maps, NT3


def kernel(X1, X0, U, W_enc, A_all, B_rest, C_w, C_b):
    in_maps, NT3 = make_in_maps(X1, X0, U, W_enc, A_all, B_rest, C_w, C_b)
    nc = _get_nc(NT3)
    res = bass_utils.run_bass_kernel_spmd(nc, in_maps, list(range(NCORES)))
    total = sum(float(r["loss_out"].sum()) for r in res.results)
    return np.float32(ALPHA * total / (WSCALE * WSCALE * ENC * N))
